# revision 40
# baseline (speedup 1.0000x reference)
"""CARAFE (content-aware upsampling) Trainium2 kernel.

Full inputs -> shard over 8 NeuronCores (batch x image-half) -> bass/Tile
kernel per core -> gather full output.

Reference semantics:
  comp = conv1x1(x, w_comp) + b_comp                    [n,64,64,64]
  mask = conv3x3(comp, w_enc, pad=1) + b_enc            [n,100,64,64]
  m    = softmax over 25 of pixel_shuffle(mask, 2)      [n,25,128,128]
  out[n,c,i,j] = sum_k m[n,k,i,j] * xpad[n,c,i//2+p, j//2+q],  k=5p+q

v2: bf16 matmul operands (1 cyc/row vs 4 for f32), SBUF-resident band
(no DRAM round trip), batched DMAs (~30 instead of 273).
"""
import numpy as np
import sys
from contextlib import ExitStack

sys.path.insert(0, "/opt/trn_rl_repo")

# ---------------- problem constants (hardcoded per spec) ----------------
N_B, C, H, W = 4, 256, 64, 64
CC = 64            # compressed channels
K5 = 5             # carafe kernel
S = 2              # scale
CM = K5 * K5 * S * S   # 100 mask channels
NCORES = 8
RH = H // 2        # 32 low-res rows per core
SLAB = RH + 4      # 36 x-rows per core (h0-2 .. h0+33)
NBLK = RH // 2     # 16 h-pair blocks
PIXC = RH * W      # 2048 low-res pixels per core
HO, WO = 2 * RH, 2 * W   # 64 x 128 output shard
NXT = SLAB // 2    # 18 non-overlapping xT row-pair tiles

_MM_DT = "bfloat16"


def _build_program():
    import concourse.bass as bass
    import concourse.tile as tile
    from concourse import bacc, mybir
    from concourse.ap import AP
    AF = mybir.ActivationFunctionType

    def pstep(t):
        return t[:].ap[0][0]

    f32 = mybir.dt.float32
    mmdt = getattr(mybir.dt, _MM_DT)

    nc = bacc.Bacc("TRN2", target_bir_lowering=False, debug=False,
                   num_devices=NCORES)

    # ---------------- DRAM parameters ----------------
    xs = nc.dram_tensor("xs", [C, SLAB, W], mmdt, kind="ExternalInput")
    xte = nc.dram_tensor("xte", [128, NXT * C], mmdt, kind="ExternalInput")
    xto = nc.dram_tensor("xto", [128, (NXT - 1) * C], mmdt, kind="ExternalInput")
    wcT = nc.dram_tensor("wcT", [C, CC], mmdt, kind="ExternalInput")
    bc = nc.dram_tensor("bc", [CC, 1], f32, kind="ExternalInput")
    weT = nc.dram_tensor("weT", [9, CC, CM], mmdt, kind="ExternalInput")
    be = nc.dram_tensor("be", [CM, 1], f32, kind="ExternalInput")
    identT = nc.dram_tensor("identT", [CM, CM + 4], mmdt, kind="ExternalInput")
    out = nc.dram_tensor("out", [C, HO, WO], mmdt, kind="ExternalOutput")

    COMP_W = W + 2      # 66: comp cols with 1 zero col each side
    HB = 2 * W * K5 * 2   # 1280 band cols per block: (a2, jl64, p5, b2)

    with tile.TileContext(nc) as tc:
        with ExitStack() as ctx:
            cpool = ctx.enter_context(tc.tile_pool(name="const", bufs=1))
            xpool = ctx.enter_context(tc.tile_pool(name="xdata", bufs=1))
            work = ctx.enter_context(tc.tile_pool(name="work", bufs=3))
            opool = ctx.enter_context(tc.tile_pool(name="oevac", bufs=2))
            pers = ctx.enter_context(tc.tile_pool(name="pers", bufs=1))
            ps_comp_p = ctx.enter_context(tc.tile_pool(name="pscomp", bufs=1, space="PSUM"))
            ps_mask_p = ctx.enter_context(tc.tile_pool(name="psmask", bufs=2, space="PSUM"))
            ps_t_p = ctx.enter_context(tc.tile_pool(name="pst", bufs=1, space="PSUM"))
            ps_o_p = ctx.enter_context(tc.tile_pool(name="pso", bufs=2, space="PSUM"))

            # ---------------- load constants ----------------
            t_wc = [cpool.tile([128, CC], mmdt, tag=f"wc{k}", name=f"wc{k}") for k in range(2)]
            for k in range(2):
                nc.sync.dma_start(t_wc[k][:], wcT.ap()[128 * k:128 * (k + 1), :])
            t_bc = cpool.tile([CC, 1], f32, tag="bc", name="bc")
            nc.sync.dma_start(t_bc[:], bc.ap())
            t_we = cpool.tile([CC, 9 * CM], mmdt, tag="we", name="we")
            src_we = AP(weT.ap().tensor, 0, [[CM, CC], [CC * CM, 9], [1, CM]])
            nc.sync.dma_start(t_we[:], src_we)
            t_be = cpool.tile([CM, 1], f32, tag="be", name="be")
            nc.sync.dma_start(t_be[:], be.ap())
            # [I_100 | sel01]: transpose-matmul rhs; cols 100..103 sum k%4
            t_id2 = cpool.tile([CM, CM + 4], mmdt, tag="id2", name="id2")
            nc.sync.dma_start(t_id2[:], identT.ap())

            # ---------------- load x ----------------
            # x_sb: 2 tiles [128, SLAB*W] channel-major
            t_x = [xpool.tile([128, SLAB * W], mmdt, tag=f"x{k}", name=f"x{k}") for k in range(2)]
            for k in range(2):
                nc.sync.dma_start(t_x[k][:], xs.ap()[128 * k:128 * (k + 1), :, :])

            # xT in two alignments so any row is available at base partition
            # 0 AND 64: xTe col-block k = rows (2k, 2k+1), k=0..17;
            # xTo col-block k = rows (2k+1, 2k+2), k=0..16.
            # Host preps these in the exact SBUF layout -> 2 full-rate DMAs.
            t_xTe = xpool.tile([128, NXT * C], mmdt, tag="xTe", name="xTe")
            t_xTo = xpool.tile([128, (NXT - 1) * C], mmdt, tag="xTo", name="xTo")
            nc.sync.dma_start(t_xTe[:], xte.ap())
            nc.sync.dma_start(t_xTo[:], xto.ap())

            # band zero-write: after xT on the queue (xT is needed first),
            # still well before the scatter needs it.
            BROW = NBLK * HB
            bndz = nc.dram_tensor("bndz", [128, BROW], mmdt)
            t_zero = pers.tile([128, HB * 2], mmdt, tag="zero", name="zero")
            nc.vector.memset(t_zero[:], 0.0)
            for zc in range(NBLK // 2):
                nc.sync.dma_start(bndz.ap()[:, HB * 2 * zc:HB * 2 * (zc + 1)],
                                  t_zero[:])

            def xrow(rho, r, ct):
                """lhsT [64, 128]: x slab-row rho at base partition 64*r."""
                if (rho % 2) == r:
                    t_xt, k = t_xTe, (rho - r) // 2
                else:
                    t_xt, k = t_xTo, (rho - 1 - r) // 2
                return t_xt[64 * r:64 * r + 64,
                            C * k + 128 * ct:C * k + 128 * (ct + 1)]

            # ---------------- comp = 1x1 conv + bias (rows 1..34 of slab) ----
            # comp stored [CC, 34, 66] with zero cols 0 and 65
            t_comp = pers.tile([CC, (RH + 2) * COMP_W], mmdt, tag="comp", name="comp")
            compv = t_comp[:].rearrange("p (r w) -> p r w", w=COMP_W)
            nc.vector.memset(compv[:, :, 0:1], 0.0)
            nc.vector.memset(compv[:, :, COMP_W - 1:COMP_W], 0.0)

            NPIX_C = (RH + 2) * W  # 2176 pixels (rows 1..34 of slab)
            ctile = 512
            nct = (NPIX_C + ctile - 1) // ctile
            for nt in range(nct):
                p0 = nt * ctile
                n = min(ctile, NPIX_C - p0)
                ps = ps_comp_p.tile([CC, ctile], f32, tag="ps_comp", name="ps_comp")
                for k in range(2):
                    rhs = AP(t_x[k][:].tensor, W + p0, [[pstep(t_x[k]), 128], [1, n]])
                    nc.tensor.matmul(ps[:, :n], t_wc[k][:], rhs,
                                     start=(k == 0), stop=(k == 1))
                r0 = p0 // W
                nr = n // W
                dst = compv[:, r0:r0 + nr, 1:1 + W]
                nc.scalar.activation(dst, ps[:, :n].rearrange("p (r w) -> p r w", w=W),
                                     func=AF.Identity, bias=t_bc[:])

            # ---------------- mask conv 3x3 -> exp ----------------
            # emask [100, RH*W]: exp(mask); Z comes out of the transpose
            t_em = pers.tile([CM, PIXC], mmdt, tag="emask", name="emask")
            emv = t_em[:].rearrange("p (r w) -> p r w", w=W)
            mtile = 512
            for nt in range(PIXC // mtile):
                mr0 = nt * mtile // W   # 8 mask rows per tile
                ps = ps_mask_p.tile([CM, mtile], f32, tag="ps_mask", name="ps_mask")
                first = True
                for dy in range(3):
                    for dx in range(3):
                        tap = dy * 3 + dx
                        rhs = compv[:, mr0 + dy:mr0 + dy + 8, dx:dx + W]
                        nc.tensor.matmul(ps[:], t_we[:, tap * CM:(tap + 1) * CM],
                                         rhs, start=first,
                                         stop=(tap == 8))
                        first = False
                # exp(mask + be) -> emask rows 0..99
                dst = emv[0:CM, mr0:mr0 + 8, :]
                nc.scalar.activation(dst, ps[:].rearrange("p (r w) -> p r w", w=W),
                                     func=AF.Exp, bias=t_be[:])

            # ---------------- mask pipeline: all blocks -> rppall ----------
            # rppall [128 pix (r, jl), 16 * 100]: block t cols (a, qd, p, b)
            # with qd = 4 - q  (so x col w' = jl + 2 - qd)
            t_rpa = pers.tile([128, NBLK * CM], mmdt, tag="rppall", name="rppall")
            rps = pstep(t_rpa)
            for t in range(NBLK):
                # "transpose" via matmul: emask[:, blk].T @ [I|sel]
                # -> [128 pix, 104]: cols 0..99 masks, 100..103 Z per ab
                psT = ps_t_p.tile([128, CM + 4], f32, tag="ps_T", name="ps_T")
                src = emv[:, 2 * t:2 * t + 2, :].rearrange("p a b -> p (a b)")
                nc.tensor.matmul(psT[:], src, t_id2[:], start=True, stop=True)

                # reciprocal of Z (4 cols, straight from PSUM)
                t_rz = work.tile([128, 4], f32, tag="rz", name="rz")
                nc.vector.reciprocal(t_rz[:], psT[:, CM:CM + 4])

                # normalize + R'' permute fused: rpp col (qd, p, a, b) =
                # psT[ch 20p+16-4qd+2a+b] * rz[2a+b]
                tps = pstep(psT)
                rzs = pstep(t_rz)
                for a in range(2):
                    in0 = AP(psT[:].tensor, 16 + 2 * a,
                             [[tps, 128], [-4, 5], [20, 5], [1, 2]])
                    in1 = AP(t_rz[:].tensor, 2 * a,
                             [[rzs, 128], [0, 5], [0, 5], [1, 2]])
                    dstp = AP(t_rpa[:].tensor, CM * t + 2 * a,
                              [[rps, 128], [20, 5], [4, 5], [1, 2]])
                    nc.vector.tensor_mul(dstp, in0, in1)

            # ---------------- band: DRAM round trip ------------------------
            # The (w' <- jl) shear is only expressible in flat DRAM
            # addressing (SBUF APs must have clean partition steps).
            # bndz flat [(r, w') 128, (t, jl, p, a, b) NBLK*HB].
            # scatter: one DMA per (qd, r) covering all (t, jl-diag, p, a, b).
            # band cols (jl, p, a, b); rpp cols (qd, p, a, b) -> inner 20 el
            # contiguous on both sides; DMA APs are limited to 3 dims.
            HBLK = NBLK // 2
            with nc.allow_non_contiguous_dma(reason="banded mask scatter"):
                for th in range(2):             # t-half: overlap with CARAFE
                    for qd in range(K5):
                        wp0 = max(0, 2 - qd)    # first valid w'
                        jl0 = max(0, qd - 2)    # = wp0 + qd - 2
                        cnt = W - abs(qd - 2)
                        for r in range(2):
                            dst = AP(bndz.ap().tensor,
                                     (64 * r + wp0) * BROW + jl0 * 20
                                     + th * HBLK * HB,
                                     [[BROW + 20, cnt],  # (w', jl) diagonal
                                      [HB, HBLK],        # t
                                      [1, 20]])          # (p, a, b)
                            srcb = AP(t_rpa[:].tensor,
                                      (64 * r + jl0) * rps + qd * 20
                                      + th * HBLK * CM,
                                      [[rps, cnt],       # jl (partition walk)
                                       [CM, HBLK],       # t
                                       [1, 20]])         # (p, a, b)
                            nc.sync.dma_start(dst, srcb)

            # bandall SBUF [128 (r, w'), NBLK * HB]: block t cols (jl, p, a, b)
            t_bnd = pers.tile([128, NBLK * HB], mmdt, tag="bandall", name="bandall")
            bps = pstep(t_bnd)

            # ---------------- CARAFE + batched output ----------------------
            # out[c_tile, (a,r,jl,b)] = sum_p xT[2t+p].T @ band_p
            GRP = 4                       # blocks per output DMA
            RBG = 2                       # blocks per band readback DMA
            for t in range(NBLK):
                g, gi = t // GRP, t % GRP
                if t % RBG == 0:
                    # readback this chunk's band: DRAM -> SBUF (dense)
                    nc.sync.dma_start(
                        t_bnd[:, HB * t:HB * (t + RBG)],
                        bndz.ap()[:, HB * t:HB * (t + RBG)])
                if gi == 0:
                    t_og = [opool.tile([128, GRP * 512], mmdt, tag=f"og{c}",
                                       name=f"og{c}g{g}") for c in range(2)]
                for ct in range(2):
                    # NOTE: two accumulation groups inside one PSUM tile
                    # crash the device; keep one PSUM tile per r.
                    pso = [ps_o_p.tile([128, 256], f32, tag=f"ps_o{rr}",
                                       name=f"ps_o{rr}") for rr in range(2)]
                    for r in range(2):
                        for p in range(K5):
                            rhs = AP(t_bnd[:].tensor,
                                     64 * r * bps + HB * t + 4 * p,
                                     [[bps, W], [2, 2],
                                      [20, W], [1, 2]])
                            nc.tensor.matmul(
                                pso[r][:],
                                xrow(2 * t + p + r, r, ct), rhs,
                                start=(p == 0), stop=(p == K5 - 1))
                    for r in range(2):
                        dst = t_og[ct][:, 512 * gi + 256 * r:
                                       512 * gi + 256 * (r + 1)]
                        if ct == 0:
                            nc.vector.tensor_copy(dst, pso[r][:])
                        else:
                            nc.scalar.activation(dst, pso[r][:], func=AF.Copy)
                if gi == GRP - 1:
                    # 16 output rows (hr = 4*(t-3) .. 4*t+3), cols contiguous
                    for ct in range(2):
                        dsto = AP(out.ap().tensor,
                                  ct * 128 * HO * WO + 4 * (t - GRP + 1) * WO,
                                  [[HO * WO, 128], [1, GRP * 512]])
                        nc.sync.dma_start(dsto, t_og[ct][:])

    nc.compile()
    return nc


_CACHE = {}


def _get_program():
    if "nc" not in _CACHE:
        _CACHE["nc"] = _build_program()
    return _CACHE["nc"]


def host_prep(x, w_comp, b_comp, w_enc, b_enc):
    """Build per-core input maps."""
    import ml_dtypes
    bf16 = ml_dtypes.bfloat16
    x = np.asarray(x, dtype=np.float32)
    wcT = np.ascontiguousarray(
        np.asarray(w_comp, np.float32).reshape(CC, C).T).astype(bf16)
    bcv = np.asarray(b_comp, np.float32).reshape(CC, 1)
    # weT[tap, cin, cout]
    weT = np.ascontiguousarray(
        np.asarray(w_enc, np.float32).reshape(CM, CC, 9).transpose(2, 1, 0)
    ).astype(bf16)
    be = np.asarray(b_enc, np.float32).reshape(CM, 1)
    identT = np.zeros((CM, CM + 4), np.float32)
    identT[np.arange(CM), np.arange(CM)] = 1.0
    identT[np.arange(CM), CM + np.arange(CM) % 4] = 1.0
    identT = identT.astype(bf16)

    in_maps = []
    for core in range(NCORES):
        n, half = core // 2, core % 2
        h0 = RH * half
        slab = np.zeros((C, SLAB, W), np.float32)
        r_lo, r_hi = h0 - 2, h0 + SLAB - 2       # x rows [r_lo, r_hi)
        v_lo, v_hi = max(0, r_lo), min(H, r_hi)
        slab[:, v_lo - r_lo:v_hi - r_lo, :] = x[n, :, v_lo:v_hi, :]
        slab16 = slab.astype(bf16)
        # xT tiles in SBUF layout: [128 part = (e, w), nk * C cols],
        # block k = x rows (2k+row0, 2k+row0+1)
        xsT = slab16.reshape(C, SLAB, W).transpose(1, 2, 0)  # [row, w, ch]
        nxt1 = SLAB // 2
        xte = np.ascontiguousarray(
            xsT.reshape(nxt1, 2, W, C).transpose(1, 2, 0, 3).reshape(
                128, nxt1 * C))
        xto = np.ascontiguousarray(
            xsT[1:SLAB - 1].reshape(nxt1 - 1, 2, W, C).transpose(
                1, 2, 0, 3).reshape(128, (nxt1 - 1) * C))
        in_maps.append({"xs": slab16, "xte": xte, "xto": xto, "wcT": wcT,
                        "bc": bcv, "weT": weT, "be": be, "identT": identT})
    return in_maps


def host_gather(results):
    out = np.empty((N_B, C, S * H, S * W), np.float32)
    for core in range(NCORES):
        n, half = core // 2, core % 2
        out[n, :, HO * half:HO * (half + 1), :] = np.asarray(
            results[core]["out"], np.float32)
    return out


def kernel(x, w_comp, b_comp, w_enc, b_enc):
    from concourse.bass_utils import run_bass_kernel_spmd
    nc = _get_program()
    in_maps = host_prep(x, w_comp, b_comp, w_enc, b_enc)
    res = run_bass_kernel_spmd(nc, in_maps, list(range(NCORES)))
    return host_gather(res.results)
